# revision 1
# baseline (speedup 1.0000x reference)
"""Trainium2 Bass kernel for nn_MultiHeadAttention_32031866093611.

Sharding: pure data parallel — batch b -> NeuronCore b (B == n_cores == 8).
Weights replicated. No collectives.

Per-core program (batch b, S=1024, D=1024, H=16, DK=64), all matmuls fp32r:

  inputs (per core): xT = x[b].T [D, S], Wq/Wk/Wv/Wo [D, D] (as stored),
                     bq/bk/bv/bo [1, D], masks (host-built from prefix[b]).

  qT[c]   = (Wq[:, c*128:+128]).T @ xT + bq       -> [128 d', 1024 s]   (8 chunks)
  kT[c]   = same with Wk                          -> [128 d', 1024 s]
  v[sc]   = (xT[:, sc*128:+128]).T @ Wv + bv      -> [128 s, 16, 64+1]  (ones col)
  per head h (c=h//2, r=h%2*64):
    for kc in 0..7:
      sT[kc] = kT[c][r:r+64, kc*128:+128].T @ qT[c][r:r+64, :]   # [128 k, 1024 q]
      sT[kc] += diag/column additive masks (DVE, on cols >= kc*128)
      eT[kc] = exp(sT[kc])                                        # ACT, psum->sbuf
      outT  += v[kc][:, h, :].T @ eT[kc]       # [65, 1024]: row 64 = softmax denom
    attnT[c][r:r+64, :] = outT[0:64, :] * bcast(1/outT[64, :])
  out[sc] = (attnT[.][:, sc*128:+128]).T @ Wo + bo  -> [128 s, 1024 d] -> DRAM

The mask allowed(q,k) = (q < prefix) | (k >= q) decomposes in the transposed
[k, q] tile grid as: blocks kc > qc fully allowed (untouched); everything at or
below the diagonal (cols >= kc*128) gets one multiplicative 0/1 u8 mask applied
to the exp output on DVE (exp(s)*m == exp(s + additive mask)).

Schedule: flat (h, kc) stream with PV matmuls lagging scores/exp by 3 tiles
(in-order PE never waits on a just-issued exp); o_proj chunk k (which only
needs heads 2k, 2k+1 after the interleave) is emitted two heads after head
2k+1 retires, inside the ACT-bound attention phase; ~20 warm-up matmuls keep
the PE HAM clock-gate hot while the first x/Wq DMAs land.
"""

import numpy as np

import concourse.bass as bass
import concourse.mybir as mybir
import concourse.tile as tile
from concourse import bacc
from concourse.bass_utils import run_bass_kernel_spmd

B, S, D, H = 8, 1024, 1024, 16
DK = D // H  # 64
P = 128
NCHUNK = S // P  # 8
NCORES = 8
F32R = mybir.dt.float32r
F32 = mybir.dt.float32
EXP = mybir.ActivationFunctionType.Exp
NEG = -1.0e30
HALF = 512  # fp32 moving-operand max
MSK_OFF = [0]
for _kc in range(1, 8):
    MSK_OFF.append(MSK_OFF[-1] + S - (_kc - 1) * P)

_CACHED = {}


def build_nc(repeats=1):
    nc = bacc.Bacc("TRN2", target_bir_lowering=False, debug=False, num_devices=NCORES)

    xt_d = nc.dram_tensor("xt", [D, S], F32R, kind="ExternalInput").ap()
    wq_d = nc.dram_tensor("wq", [D, D], F32R, kind="ExternalInput").ap()
    wk_d = nc.dram_tensor("wk", [D, D], F32R, kind="ExternalInput").ap()
    wv_d = nc.dram_tensor("wv", [D, D], F32R, kind="ExternalInput").ap()
    wo_d = nc.dram_tensor("wo", [D, D], F32R, kind="ExternalInput").ap()
    bqk_d = nc.dram_tensor("bqk", [P, 2 * NCHUNK], F32, kind="ExternalInput").ap()
    ones_d = nc.dram_tensor("ones2d", [P, P], F32R, kind="ExternalInput").ap()
    bv_d = nc.dram_tensor("bv", [P, D], F32, kind="ExternalInput").ap()
    bo_d = nc.dram_tensor("bo", [P, D], F32, kind="ExternalInput").ap()
    msk_d = nc.dram_tensor("mask8", [P, 4608], mybir.dt.uint8, kind="ExternalInput").ap()
    out_d = nc.dram_tensor("out", [S, D], F32, kind="ExternalOutput").ap()

    with tile.TileContext(nc) as tc:
        with (
            tc.tile_pool(name="w", bufs=18) as wpool,
            tc.tile_pool(name="big", bufs=2) as bigpool,
            tc.tile_pool(name="qk", bufs=8) as qkpool,
            tc.tile_pool(name="v", bufs=8) as vpool,
            tc.tile_pool(name="cst", bufs=1) as cstpool,
            tc.tile_pool(name="exp", bufs=5) as exppool,
            tc.tile_pool(name="rcp", bufs=1) as rcppool,
            tc.tile_pool(name="rbc", bufs=1) as rbcpool,
            tc.tile_pool(name="osb", bufs=1) as osbpool,
            tc.tile_pool(name="pp", bufs=2, space="PSUM") as pp,
            tc.tile_pool(name="po", bufs=2, space="PSUM") as po,
        ):
            for _rep in range(repeats):
                # ---- x chunks + Wq strips interleaved (fast PE start), cst after ----
                ones2d = cstpool.tile([P, P], F32R, tag="ones2d")
                nc.sync.dma_start(ones2d[:], ones_d[:])
                ones = ones2d[0:1, :]
                xtq = [
                    bigpool.tile([P, 4, S], F32R, tag="big", name=f"xtq_{g}")
                    for g in range(2)
                ]
                def whalf(nm, w_dram, hf):
                    """8 half-strips [128, 512] of W columns [hf*512, (hf+1)*512)."""
                    ts = [
                        wpool.tile([P, HALF], F32R, tag="w", name=f"{nm}{hf}_{dc}")
                        for dc in range(NCHUNK)
                    ]
                    sl = slice(hf * HALF, (hf + 1) * HALF)
                    for dc in range(NCHUNK):
                        nc.sync.dma_start(ts[dc][:], w_dram[dc * P : (dc + 1) * P, sl])
                    return ts

                for dc in range(NCHUNK):
                    nc.sync.dma_start(
                        xtq[dc // 4][:, dc % 4, 0:HALF],
                        xt_d[dc * P : (dc + 1) * P, 0:HALF],
                    )
                qh0 = whalf("wq", wq_d, 0)
                for dc in range(NCHUNK):
                    nc.sync.dma_start(
                        xtq[dc // 4][:, dc % 4, HALF:S],
                        xt_d[dc * P : (dc + 1) * P, HALF:S],
                    )
                kh0 = whalf("wk", wk_d, 0)
                xt = [xtq[dc // 4][:, dc % 4, :] for dc in range(NCHUNK)]

                # PE warm-up: ~20 throwaway matmuls on the first-arriving tiny
                # tile keep the HAM clock-gate busy while x/Wq stream in.
                wps = pp.tile([P, S], F32, tag="pp", name="warmup_ps")
                for wi in range(18):
                    nc.tensor.matmul(
                        wps[:, 0:P], ones2d[:], ones2d[:], start=True, stop=True
                    )
                bqk = cstpool.tile([P, 2 * NCHUNK], F32, tag="bqk")
                nc.sync.dma_start(bqk[:], bqk_d[:])
                msk = cstpool.tile([P, 4608], mybir.dt.uint8, tag="msk")
                nc.sync.dma_start(msk[:], msk_d[:])
                bias = {}
                # bv (v-proj) and bo (o_proj) lifetimes don't overlap: share slot
                bias["bv"] = cstpool.tile([P, D], F32, tag="bvbo", name="bv_bc")
                nc.sync.dma_start(bias["bv"][:], bv_d[:])

                # ---- helper: dense [d', s] projection (qT / kT) ----
                def proj_half(whalf_tiles, chalf, bcol0, out_tag):
                    """qT/kT chunks chalf*4 .. chalf*4+3 from one W column half."""
                    outs = []
                    for cp in range(2):
                        cs = (chalf * 4 + 2 * cp, chalf * 4 + 2 * cp + 1)
                        pss = {
                            c: pp.tile([P, S], F32, tag="pp", name=f"ps_{out_tag}_{c}")
                            for c in cs
                        }
                        for j in range(2):
                            sl = slice(j * HALF, (j + 1) * HALF)
                            for c in cs:
                                lc = (c % 4) * P
                                for dc in range(NCHUNK):
                                    nc.tensor.matmul(
                                        pss[c][:, sl],
                                        whalf_tiles[dc][:, lc : lc + P],
                                        xt[dc][:, sl],
                                        start=(dc == 0),
                                        stop=(dc == NCHUNK - 1),
                                    )
                        for c in cs:
                            o = qkpool.tile(
                                [P, S], F32R, tag=out_tag, name=f"{out_tag}_{c}"
                            )
                            nc.vector.tensor_add(
                                o[:],
                                pss[c][:],
                                bqk[:, bcol0 + c : bcol0 + c + 1].to_broadcast((P, S)),
                            )
                            outs.append(o)
                    return outs

                with nc.named_scope("qk_proj"):
                    qT = proj_half(qh0, 0, 0, "qT")
                    qh1 = whalf("wq", wq_d, 1)
                    kT = proj_half(kh0, 0, NCHUNK, "kT")
                    kh1 = whalf("wk", wk_d, 1)
                    qT += proj_half(qh1, 1, 0, "qT")
                    kT += proj_half(kh1, 1, NCHUNK, "kT")

                # ---- v projection: [s, 16, 65] with ones column ----
                with nc.named_scope("v_proj"):
                    vh = [whalf("wv", wv_d, 0), whalf("wv", wv_d, 1)]
                    vtiles = []
                    for sc in range(NCHUNK):
                        ps = pp.tile([P, S], F32, tag="pp")
                        for j in range(2):
                            sl = slice(j * HALF, (j + 1) * HALF)
                            for dc in range(NCHUNK):
                                nc.tensor.matmul(
                                    ps[:, sl],
                                    xt[dc][:, sc * P : (sc + 1) * P],
                                    vh[j][dc][:],
                                    start=(dc == 0),
                                    stop=(dc == NCHUNK - 1),
                                )
                        vt = vpool.tile([P, H, DK + 1], F32R, tag="v")
                        nc.vector.tensor_add(
                            vt[:, :, 0:DK],
                            ps[:].rearrange("p (h d) -> p h d", h=H),
                            bias["bv"][:].rearrange("p (h d) -> p h d", h=H),
                        )
                        nc.vector.tensor_copy(
                            vt[:, :, DK : DK + 1], ones2d[:, 0:1].to_broadcast((P, H, 1))
                        )
                        vtiles.append(vt)

                # ---- attention heads ----
                bias["bo"] = cstpool.tile([P, D], F32, tag="bvbo", name="bo_bc")
                nc.sync.dma_start(bias["bo"][:], bo_d[:])
                attn = [None, None]

                # Wo strips prefetched before the head loop (slots free up as
                # Wq/Wk strips retire); o_proj chunk sc only needs heads 2sc,2sc+1.
                oh = [whalf("wo", wo_d, 0), whalf("wo", wo_d, 1)]

                def emit_scores_exp(h, kc):
                    """scores on PE, exp on ACT, multiplicative 0/1 masks on DVE."""
                    c, r = h // 2, (h % 2) * DK
                    pss = pp.tile([P, S], F32, tag="pp", name=f"pss_{h}_{kc}")
                    lhs = kT[c][r : r + DK, kc * P : (kc + 1) * P]
                    for j in range(2):
                        sl = slice(j * HALF, (j + 1) * HALF)
                        nc.tensor.matmul(
                            pss[:, sl],
                            lhs,
                            qT[c][r : r + DK, sl],
                            start=True,
                            stop=True,
                        )
                    et = exppool.tile([P, S], F32R, tag="exp", name=f"et_{h}_{kc}")
                    nc.scalar.activation(et[:], pss[:], EXP)
                    # one 0/1 mask mult over cols [kc*128, 1024): diag pattern on
                    # the diagonal block, column mask below the diagonal
                    w = S - kc * P
                    off = MSK_OFF[kc]
                    nc.vector.tensor_mul(
                        et[:, kc * P : S], et[:, kc * P : S], msk[:, off : off + w]
                    )
                    return et

                def emit_pv(h, kc, pso, et):
                    for j in range(2):
                        sl = slice(j * HALF, (j + 1) * HALF)
                        nc.tensor.matmul(
                            pso[0 : DK + 1, sl],
                            vtiles[kc][:, h, :],
                            et[:, sl],
                            start=(kc == 0),
                            stop=(kc == NCHUNK - 1),
                        )

                def emit_norm(h, pso):
                    rcp = rcppool.tile([1, S], F32, tag="rcp", name=f"rcp_{h}")
                    nc.vector.reciprocal(rcp[:], pso[DK : DK + 1, :])
                    rbc = rbcpool.tile([DK, S], F32, tag="rbc", name=f"rbc_{h}")
                    nc.gpsimd.partition_broadcast(rbc[:], rcp[:])
                    # attn[g][e*64+d, cc, h*64+u] = O_h[u*16 + 2*(4g+cc) + e, d]/denom
                    src = pso[0:DK, :].rearrange("d (u j) -> d j u", j=16)
                    rbs = rbc[:].rearrange("d (u j) -> d j u", j=16)
                    for g in range(2):
                        if attn[g] is None:
                            attn[g] = bigpool.tile(
                                [P, 4, S], F32R, tag="big", name=f"attnq_{g}"
                            )
                        for e in range(2):
                            jsl = slice(8 * g + e, 8 * (g + 1), 2)
                            nc.vector.tensor_mul(
                                attn[g][e * DK : (e + 1) * DK, :, h * DK : (h + 1) * DK],
                                src[:, jsl, :],
                                rbs[:, jsl, :],
                            )

                def emit_oproj(sc):
                    ps = po.tile([P, S], F32, tag="po", name=f"psf_{sc}")
                    for j in range(2):
                        sl = slice(j * HALF, (j + 1) * HALF)
                        for cc in range(NCHUNK):
                            nc.tensor.matmul(
                                ps[:, sl],
                                attn[cc // 4][:, cc % 4, sc * P : (sc + 1) * P],
                                oh[j][cc][:],
                                start=(cc == 0),
                                stop=(cc == NCHUNK - 1),
                            )
                    ot = osbpool.tile([P, S], F32, tag="osb", name=f"ot_{sc}")
                    nc.vector.tensor_add(ot[:], ps[:], bias["bo"][:])
                    nc.sync.dma_start(out_d[sc * P : (sc + 1) * P, :], ot[:])

                # Flat (h, kc) stream, PV lagging scores/exp by one tile so the
                # in-order PE never waits on a just-issued exp. After the last
                # PV of a head, the accumulator is copied to SBUF immediately to
                # free its PSUM bank; the norm chain reads the copy. o_proj
                # chunk k (needs heads 2k,2k+1 only) is emitted two heads later.
                from collections import deque
                pend = deque()
                pso_cur = None

                def pop_pv():
                    ph, pkc, ppso, pet = pend.popleft()
                    emit_pv(ph, pkc, ppso, pet)
                    if pkc == NCHUNK - 1:
                        emit_norm(ph, ppso)
                        if ph % 2 == 1 and ph >= 3:
                            emit_oproj((ph - 3) // 2)

                for h in range(H):
                    pso_cur = po.tile([P, S], F32, tag="po", name=f"pso_{h}")
                    for kc in range(NCHUNK):
                        et = emit_scores_exp(h, kc)
                        if len(pend) >= 4:
                            pop_pv()
                        pend.append((h, kc, pso_cur, et))
                while len(pend) > 1:
                    pop_pv()
                # last PV of head 15: slot o_proj(6) in front of the norm chain
                # so the PE stays busy while recip/bcast run on DVE/Pool.
                ph, pkc, ppso, pet = pend.popleft()
                emit_pv(ph, pkc, ppso, pet)
                emit_oproj(NCHUNK - 2)
                emit_norm(ph, ppso)
                emit_oproj(NCHUNK - 1)

    nc.compile()
    return nc


def _host_masks(prefix_b: int):
    """Combined multiplicative 0/1 mask, u8, applied to exp output.

    For scores-T tile kc (cols q in [kc*128, 1024)): element (i, q) keeps
    exp iff allowed(q, k=kc*128+i) = (q < prefix) or (k >= q).
    Segment kc occupies msk[:, off_kc : off_kc + (1024 - kc*128)].
    """
    i = np.arange(P)[:, None]
    segs = []
    for kc in range(NCHUNK):
        q = np.arange(kc * P, S)[None, :]
        k = kc * P + i
        allowed = (q < prefix_b) | (k >= q)
        segs.append(allowed.astype(np.uint8))
    return np.concatenate(segs, axis=1)


def kernel(x, prefix, Wq, bq, Wk, bk, Wv, bv, Wo, bo, _trace=False):
    x = np.asarray(x, dtype=np.float32)
    prefix = np.asarray(prefix)
    Wq, Wk, Wv, Wo = (np.ascontiguousarray(np.asarray(w, np.float32)) for w in (Wq, Wk, Wv, Wo))
    bv, bo = (
        np.broadcast_to(np.asarray(v, np.float32).reshape(1, D), (P, D)).copy()
        for v in (bv, bo)
    )
    bqk = np.stack(
        [np.asarray(bq, np.float32).reshape(NCHUNK, P), np.asarray(bk, np.float32).reshape(NCHUNK, P)], axis=0
    ).reshape(2 * NCHUNK, P).T.copy()  # [128, 16]: cols 0-7 = bq chunks, 8-15 = bk

    ones2d = np.ones((P, P), dtype=np.float32)
    if "nc" not in _CACHED:
        _CACHED["nc"] = build_nc()
    nc = _CACHED["nc"]

    in_maps = []
    for b in range(B):
        mask8 = _host_masks(int(prefix[b]))
        in_maps.append(
            {
                "xt": np.ascontiguousarray(x[b].T),
                "wq": Wq, "wk": Wk, "wv": Wv, "wo": Wo,
                "bqk": bqk, "bv": bv, "bo": bo, "ones2d": ones2d,
                "mask8": mask8,
            }
        )

    res = run_bass_kernel_spmd(nc, in_maps, core_ids=list(range(NCORES)), trace=_trace)
    out = np.stack([res.results[b]["out"] for b in range(B)], axis=0)
    if _trace:
        return out, res
    return out



# revision 16
# speedup vs baseline: 1.1415x; 1.1415x over previous
"""Trainium2 Bass kernel for nn_MultiHeadAttention_32031866093611.

Sharding: pure data parallel - batch b -> NeuronCore b (B == n_cores == 8).
Weights replicated. No collectives.

Design (vs fp32r baseline at 282us):
  - q/k/v projections in fp8e4m3 with hi/lo error compensation
    (x = xhi + xlo, W*32 = Whi + Wlo; q ~= xhi@Whi + xhi@Wlo + xlo@Whi) using
    DoubleRow perf mode (2 contraction tiles per matmul at 0.5 cycles/row):
    24 matmuls/chunk at ~107ns vs 16 at ~213ns -> 0.75x PE time. The 1/32
    rescale + bias fold into the PSUM->SBUF copy.
  - qT/kT/et/v/attn/Wo bf16: halves SBUF/DMA, 2x DVE on mask multiplies.
  - v tiles are [128, 16, 128] with ones in cols 64:127, so the PV matmul
    itself replicates the softmax denominator across PSUM rows 64:128
    (matmul cost is free-size only) - the norm is then one partition-shifted
    DVE reciprocal + 4 strided multiplies, no gpsimd broadcast.
  - attention pairs interleave with next-chunk q/k projections and o-proj;
    PV pops carry across pair boundaries to hide the exp latency at pair
    starts.
"""

import numpy as np
import ml_dtypes

import concourse.bass as bass
import concourse.mybir as mybir
import concourse.tile as tile
from concourse import bacc
from concourse.bass_utils import run_bass_kernel_spmd

B, S, D, H = 8, 1024, 1024, 16
DK = D // H  # 64
P = 128
NCHUNK = S // P  # 8
NCORES = 8
F32 = mybir.dt.float32
BF16 = mybir.dt.bfloat16
FP8 = mybir.dt.float8e4
EXP = mybir.ActivationFunctionType.Exp
COPY = mybir.ActivationFunctionType.Copy
MULT = mybir.AluOpType.mult
ADD = mybir.AluOpType.add
DR = mybir.MatmulPerfMode.DoubleRow
HALF = 512
WSC = 32.0  # host scales W by 32 so fp8 sees ~unit-variance values
MSK_OFF = [0]
for _kc in range(1, 8):
    MSK_OFF.append(MSK_OFF[-1] + S - (_kc - 1) * P)

_CACHED = {}


def build_nc(repeats=1):
    nc = bacc.Bacc("TRN2", target_bir_lowering=False, debug=False, num_devices=NCORES)

    xhi_d = nc.dram_tensor("xhi", [P, NCHUNK, S], FP8, kind="ExternalInput").ap()
    xlo_d = nc.dram_tensor("xlo", [P, NCHUNK, S], FP8, kind="ExternalInput").ap()
    wq8_d = {
        hl: nc.dram_tensor(f"wq8{hl}", [NCHUNK, P, 4, 2, P], FP8, kind="ExternalInput").ap()
        for hl in ("h", "l")
    }
    wk8_d = {
        hl: nc.dram_tensor(f"wk8{hl}", [NCHUNK, P, 4, 2, P], FP8, kind="ExternalInput").ap()
        for hl in ("h", "l")
    }
    wv8_d = {
        hl: nc.dram_tensor(f"wv8{hl}", [P, 4, 2, S], FP8, kind="ExternalInput").ap()
        for hl in ("h", "l")
    }
    wo_d = nc.dram_tensor("wo16", [P, NCHUNK, S], BF16, kind="ExternalInput").ap()
    bqk_d = nc.dram_tensor("bqk", [P, 2 * NCHUNK], F32, kind="ExternalInput").ap()
    bv_d = nc.dram_tensor("bv16", [P, D], BF16, kind="ExternalInput").ap()
    bo_d = nc.dram_tensor("bo32", [P, D], F32, kind="ExternalInput").ap()
    msk_d = nc.dram_tensor("msk16", [P, 4608], BF16, kind="ExternalInput").ap()
    out_d = nc.dram_tensor("out", [S, D], F32, kind="ExternalOutput").ap()

    with tile.TileContext(nc) as tc:
        with (
            tc.tile_pool(name="cst", bufs=1) as cstpool,
            tc.tile_pool(name="qk", bufs=3) as qkpool,
            tc.tile_pool(name="v", bufs=8) as vpool,
            tc.tile_pool(name="exp", bufs=6) as exppool,
            tc.tile_pool(name="rbc", bufs=2) as rbcpool,
            tc.tile_pool(name="osb", bufs=2) as osbpool,
            tc.tile_pool(name="big", bufs=2) as bigpool,
            tc.tile_pool(name="pp", bufs=2, space="PSUM") as pp,
            tc.tile_pool(name="po", bufs=2, space="PSUM") as po,
        ):
            for _rep in range(repeats):
                # ---- PE warm-up (no DMA dependency) ----
                warm = cstpool.tile([P, P], BF16, tag="warm")
                nc.vector.memzero(warm[:])
                wps = pp.tile([P, S], F32, tag="pp", name="warmup_ps")
                for wi in range(26):
                    nc.tensor.matmul(wps[:, 0:P], warm[:], warm[:], start=True, stop=True)

                # ---- DMAs, ordered for earliest dependency release ----
                xhi = cstpool.tile([P, NCHUNK, S], FP8, tag="xhi")
                nc.sync.dma_start(xhi[:, 0:4], xhi_d[:, 0:4])
                wq8 = {
                    hl: cstpool.tile([P, NCHUNK, 4, 2, P], FP8, tag=f"wq8{hl}", name=f"wq8{hl}")
                    for hl in ("h", "l")
                }
                wk8 = {
                    hl: cstpool.tile([P, NCHUNK, 4, 2, P], FP8, tag=f"wk8{hl}", name=f"wk8{hl}")
                    for hl in ("h", "l")
                }
                wv8 = {
                    hl: cstpool.tile([P, 4, 2, S], FP8, tag=f"wv8{hl}", name=f"wv8{hl}")
                    for hl in ("h", "l")
                }
                for hl in ("h", "l"):
                    nc.sync.dma_start(wq8[hl][:, 0], wq8_d[hl][0])
                nc.sync.dma_start(xhi[:, 4:8], xhi_d[:, 4:8])
                for hl in ("h", "l"):
                    nc.sync.dma_start(wk8[hl][:, 0], wk8_d[hl][0])
                nc.sync.dma_start(wv8["h"][:], wv8_d["h"][:])
                xlo = cstpool.tile([P, NCHUNK, S], FP8, tag="xlo")
                nc.sync.dma_start(xlo[:, 0:4], xlo_d[:, 0:4])
                nc.sync.dma_start(xlo[:, 4:8], xlo_d[:, 4:8])
                nc.sync.dma_start(wv8["l"][:], wv8_d["l"][:])
                bqk = cstpool.tile([P, 2 * NCHUNK], F32, tag="bqk")
                nc.sync.dma_start(bqk[:], bqk_d[:])
                bv16 = cstpool.tile([P, D], BF16, tag="bv16")
                nc.sync.dma_start(bv16[:], bv_d[:])
                msk = cstpool.tile([P, 4608], BF16, tag="msk")
                nc.sync.dma_start(msk[:], msk_d[:])
                wo16 = cstpool.tile([P, NCHUNK, S], BF16, tag="wo16")
                nc.sync.dma_start(wo16[:], wo_d[:])
                bo32 = cstpool.tile([P, D], F32, tag="bo32")
                nc.sync.dma_start(bo32[:], bo_d[:])

                def dma_wqk_cb(c):
                    for tl, dr in ((wq8, wq8_d), (wk8, wk8_d)):
                        for hl in ("h", "l"):
                            nc.sync.dma_start(tl[hl][:, c], dr[hl][c])

                dma_wqk_cb(1)

                # ---- fp8 DoubleRow projection helpers ----
                def proj_qk(c, w8, bcol, out_tag):
                  with nc.named_scope(f"pj_{out_tag}_{c}"):
                    ps = pp.tile([P, S], F32, tag="pp", name=f"ps_{out_tag}_{c}")
                    for hf in range(2):
                        sl = slice(hf * HALF, (hf + 1) * HALF)
                        seq = [(w8["h"][:, c, j], xhi[:, 2 * j : 2 * j + 2, sl]) for j in range(4)]
                        seq += [(w8["l"][:, c, j], xhi[:, 2 * j : 2 * j + 2, sl]) for j in range(4)]
                        seq += [(w8["h"][:, c, j], xlo[:, 2 * j : 2 * j + 2, sl]) for j in range(4)]
                        for i, (l, r) in enumerate(seq):
                            nc.tensor.matmul(
                                ps[:, sl], l, r,
                                start=(i == 0), stop=(i == len(seq) - 1), perf_mode=DR,
                            )
                    o = qkpool.tile([P, S], BF16, tag=out_tag, name=f"{out_tag}_{c}")
                    nc.vector.tensor_scalar(o[:], ps[:], 1.0 / WSC, bqk[:, bcol + c : bcol + c + 1], MULT, ADD)
                    return o

                def proj_v(sc):
                  with nc.named_scope(f"pj_v_{sc}"):
                    ps = pp.tile([P, S], F32, tag="pp", name=f"ps_v_{sc}")
                    ssl = slice(sc * P, (sc + 1) * P)
                    for hf in range(2):
                        sl = slice(hf * HALF, (hf + 1) * HALF)
                        seq = [(xhi[:, 2 * j : 2 * j + 2, ssl], wv8["h"][:, j, :, sl]) for j in range(4)]
                        seq += [(xhi[:, 2 * j : 2 * j + 2, ssl], wv8["l"][:, j, :, sl]) for j in range(4)]
                        seq += [(xlo[:, 2 * j : 2 * j + 2, ssl], wv8["h"][:, j, :, sl]) for j in range(4)]
                        for i, (l, r) in enumerate(seq):
                            nc.tensor.matmul(
                                ps[:, sl], l, r,
                                start=(i == 0), stop=(i == len(seq) - 1), perf_mode=DR,
                            )
                    vt = vpool.tile([P, H, P], BF16, tag="v", name=f"v_{sc}")
                    # ones in cols 64:128 -> PV psum rows 64:128 hold the
                    # denominator replicated (free: matmul cost is free-size)
                    nc.gpsimd.memset(vt[:], 1.0)
                    for hf in range(2):
                        sl = slice(hf * HALF, (hf + 1) * HALF)
                        nc.scalar.activation(
                            vt[:, hf * 8 : (hf + 1) * 8, 0:DK],
                            ps[:, sl].rearrange("p (h d) -> p h d", h=8),
                            COPY, scale=1.0 / WSC,
                        )
                    nc.vector.tensor_add(
                        vt[:, :, 0:DK],
                        vt[:, :, 0:DK],
                        bv16[:].rearrange("p (h d) -> p h d", h=H),
                    )
                    return vt

                # ---- attention pieces ----
                qT, kT, vtiles = [None] * NCHUNK, [None] * NCHUNK, [None] * NCHUNK
                attn = [None, None]

                def scores_exp(h, kc):
                    nm = nc.named_scope(f"sc_{h}_{kc}")
                    nm.__enter__()
                    c, r = h // 2, (h % 2) * DK
                    ps = pp.tile([P, S], F32, tag="pp", name=f"pss_{h}_{kc}")
                    lhs = kT[c][r : r + DK, kc * P : (kc + 1) * P]
                    for hf in range(2):
                        sl = slice(hf * HALF, (hf + 1) * HALF)
                        nc.tensor.matmul(ps[:, sl], lhs, qT[c][r : r + DK, sl], start=True, stop=True)
                    et = exppool.tile([P, S], BF16, tag="et", name=f"et_{h}_{kc}")
                    nc.scalar.activation(et[:], ps[:], EXP)
                    w = S - kc * P
                    off = MSK_OFF[kc]
                    nc.vector.tensor_mul(et[:, kc * P : S], et[:, kc * P : S], msk[:, off : off + w])
                    nm.__exit__(None, None, None)
                    return et

                def emit_pv(h, kc, pso, et):
                  with nc.named_scope(f"pv_{h}_{kc}"):
                    for hf in range(2):
                        sl = slice(hf * HALF, (hf + 1) * HALF)
                        nc.tensor.matmul(
                            pso[:, sl], vtiles[kc][:, h, :], et[:, sl],
                            start=(kc == 0), stop=(kc == NCHUNK - 1),
                        )

                def emit_norm(h, pso):
                  with nc.named_scope(f"norm_{h}"):
                    rbc = rbcpool.tile([DK, S], F32, tag="rbc", name=f"rbc_{h}")
                    nc.vector.reciprocal(rbc[:], pso[DK : 2 * DK, :])
                    src = pso[0:DK, :].rearrange("d (u j) -> d j u", j=16)
                    rbs = rbc[:].rearrange("d (u j) -> d j u", j=16)
                    for g in range(2):
                        if attn[g] is None:
                            attn[g] = bigpool.tile([P, 4, S], BF16, tag="big", name=f"attnq_{g}")
                        for e in range(2):
                            jsl = slice(8 * g + e, 8 * (g + 1), 2)
                            nc.vector.tensor_mul(
                                attn[g][e * DK : (e + 1) * DK, :, h * DK : (h + 1) * DK],
                                src[:, jsl, :], rbs[:, jsl, :],
                            )

                def emit_oproj(sc):
                  with nc.named_scope(f"oproj_{sc}"):
                    ps = po.tile([P, S], F32, tag="po", name=f"psf_{sc}")
                    for hf in range(2):
                        sl = slice(hf * HALF, (hf + 1) * HALF)
                        for cc in range(NCHUNK):
                            nc.tensor.matmul(
                                ps[:, sl],
                                attn[cc // 4][:, cc % 4, sc * P : (sc + 1) * P],
                                wo16[:, cc, sl],
                                start=(cc == 0), stop=(cc == NCHUNK - 1),
                            )
                    ot = osbpool.tile([P, S], F32, tag="osb", name=f"ot_{sc}")
                    for hf in range(2):
                        sl = slice(hf * HALF, (hf + 1) * HALF)
                        nc.vector.tensor_add(ot[:, sl], ps[:, sl], bo32[:, sl])
                        nc.sync.dma_start(out_d[sc * P : (sc + 1) * P, sl], ot[:, sl])

                # ---- pre-phase: qk chunk 0 + first v chunks ----
                qT[0] = proj_qk(0, wq8, 0, "qT")
                kT[0] = proj_qk(0, wk8, NCHUNK, "kT")
                for sc in range(4):
                    vtiles[sc] = proj_v(sc)

                # ---- pair loop (pend carried across pair boundaries) ----
                from collections import deque

                pend = deque()

                def pop_pv():
                    ph, pkc, ppso, pet = pend.popleft()
                    emit_pv(ph, pkc, ppso, pet)
                    if pkc == NCHUNK - 1:
                        emit_norm(ph, ppso)

                for c in range(NCHUNK):
                    h0, h1 = 2 * c, 2 * c + 1
                    if c < NCHUNK - 2:
                        dma_wqk_cb(c + 2)
                    pso0 = po.tile([P, S], F32, tag="po", name=f"pso_{h0}")
                    for kc in range(NCHUNK):
                        et = scores_exp(h0, kc)
                        if c == 0 and kc % 2 == 1:
                            vtiles[4 + kc // 2] = proj_v(4 + kc // 2)
                        # eagerly drain PVs of previous heads; keep lag 4 for own
                        if len(pend) >= 4 or (pend and pend[0][0] < h0):
                            pop_pv()
                        pend.append((h0, kc, pso0, et))
                    if c == NCHUNK - 1:
                        emit_oproj(NCHUNK - 2)
                    pso1 = po.tile([P, S], F32, tag="po", name=f"pso_{h1}")
                    for kc in range(NCHUNK):
                        et = scores_exp(h1, kc)
                        if len(pend) >= 4 or (pend and pend[0][0] < h0):
                            pop_pv()
                        pend.append((h1, kc, pso1, et))
                    if c < NCHUNK - 1:
                        qT[c + 1] = proj_qk(c + 1, wq8, 0, "qT")
                        pop_pv()
                        pop_pv()
                        kT[c + 1] = proj_qk(c + 1, wk8, NCHUNK, "kT")
                        while len(pend) > 3:
                            pop_pv()
                        if c >= 1:
                            emit_oproj(c - 1)
                    else:
                        while pend:
                            pop_pv()
                emit_oproj(NCHUNK - 1)

    nc.compile()
    return nc


def _host_masks(prefix_b: int):
    """Multiplicative 0/1 mask (bf16) applied to exp output."""
    i = np.arange(P)[:, None]
    segs = []
    for kc in range(NCHUNK):
        q = np.arange(kc * P, S)[None, :]
        k = kc * P + i
        allowed = (q < prefix_b) | (k >= q)
        segs.append(allowed.astype(np.float32))
    return np.concatenate(segs, axis=1).astype(ml_dtypes.bfloat16)


def _split8(a):
    hi = a.astype(ml_dtypes.float8_e4m3fn)
    lo = (a - hi.astype(np.float32)).astype(ml_dtypes.float8_e4m3fn)
    return hi, lo


def _pack_wqk(w):
    """[8cb, 128k, 4j, 2t, 128m] from W32 [(2j+t)*128+k, cb*128+m]."""
    a = (w * WSC).reshape(4, 2, P, NCHUNK, P).transpose(3, 2, 0, 1, 4)
    return _split8(np.ascontiguousarray(a))


def _pack_wv(w):
    """[128k, 4j, 2t, 1024n] from Wv32 [(2j+t)*128+k, n]."""
    a = (w * WSC).reshape(4, 2, P, S).transpose(2, 0, 1, 3)
    return _split8(np.ascontiguousarray(a))


def kernel(x, prefix, Wq, bq, Wk, bk, Wv, bv, Wo, bo, _trace=False):
    x = np.asarray(x, dtype=np.float32)
    prefix = np.asarray(prefix)
    Wq, Wk, Wv, Wo = (np.asarray(w, np.float32) for w in (Wq, Wk, Wv, Wo))
    bqk = np.stack(
        [np.asarray(bq, np.float32).reshape(NCHUNK, P), np.asarray(bk, np.float32).reshape(NCHUNK, P)],
        axis=0,
    ).reshape(2 * NCHUNK, P).T.copy()  # [128, 16]: cols 0-7 bq chunks, 8-15 bk

    wq8h, wq8l = _pack_wqk(Wq)
    wk8h, wk8l = _pack_wqk(Wk)
    wv8h, wv8l = _pack_wv(Wv)
    wo16 = np.ascontiguousarray(
        Wo.reshape(NCHUNK, P, S).transpose(1, 0, 2)
    ).astype(ml_dtypes.bfloat16)
    bv16 = np.broadcast_to(np.asarray(bv, np.float32).reshape(1, D), (P, D)).astype(ml_dtypes.bfloat16)
    bo32 = np.broadcast_to(np.asarray(bo, np.float32).reshape(1, D), (P, D)).astype(np.float32).copy()

    if "nc" not in _CACHED:
        _CACHED["nc"] = build_nc()
    nc = _CACHED["nc"]

    in_maps = []
    for b in range(B):
        xt = np.ascontiguousarray(x[b].T)  # [D, S]
        xts = np.ascontiguousarray(xt.reshape(NCHUNK, P, S).transpose(1, 0, 2))  # [128, 8, 1024]
        xhi, xlo = _split8(xts)
        mask16 = _host_masks(int(prefix[b]))
        in_maps.append(
            {
                "xhi": xhi, "xlo": xlo,
                "wq8h": wq8h, "wq8l": wq8l,
                "wk8h": wk8h, "wk8l": wk8l,
                "wv8h": wv8h, "wv8l": wv8l,
                "wo16": wo16, "bqk": bqk, "bv16": bv16, "bo32": bo32,
                "msk16": mask16,
            }
        )

    res = run_bass_kernel_spmd(nc, in_maps, core_ids=list(range(NCORES)), trace=_trace)
    out = np.stack([res.results[b]["out"] for b in range(B)], axis=0)
    if _trace:
        return out, res
    return out


# revision 23
# speedup vs baseline: 1.1438x; 1.0021x over previous
"""Trainium2 Bass kernel for nn_MultiHeadAttention_32031866093611.

Sharding: pure data parallel - batch b -> NeuronCore b (B == n_cores == 8).
Weights replicated. No collectives.

Design (vs fp32r baseline at 282us):
  - q/k/v projections in fp8e4m3 with hi/lo error compensation
    (x = xhi + xlo, W*32 = Whi + Wlo; q ~= xhi@Whi + xhi@Wlo + xlo@Whi) using
    DoubleRow perf mode (2 contraction tiles per matmul at 0.5 cycles/row):
    24 matmuls/chunk at ~107ns vs 16 at ~213ns -> 0.75x PE time. The 1/32
    rescale + bias fold into the PSUM->SBUF copy.
  - qT/kT/et/v/attn/Wo bf16: halves SBUF/DMA, 2x DVE on mask multiplies.
  - v tiles are [128, 16, 128] with ones in cols 64:127, so the PV matmul
    itself replicates the softmax denominator across PSUM rows 64:128
    (matmul cost is free-size only) - the norm is then one partition-shifted
    DVE reciprocal + 4 strided multiplies, no gpsimd broadcast.
  - attention pairs interleave with next-chunk q/k projections and o-proj;
    PV pops carry across pair boundaries to hide the exp latency at pair
    starts.
"""

import numpy as np
import ml_dtypes

import concourse.bass as bass
import concourse.mybir as mybir
import concourse.tile as tile
from concourse import bacc
from concourse.bass_utils import run_bass_kernel_spmd

B, S, D, H = 8, 1024, 1024, 16
DK = D // H  # 64
P = 128
NCHUNK = S // P  # 8
NCORES = 8
F32 = mybir.dt.float32
F32R = mybir.dt.float32r
BF16 = mybir.dt.bfloat16
FP8 = mybir.dt.float8e4
EXP = mybir.ActivationFunctionType.Exp
COPY = mybir.ActivationFunctionType.Copy
MULT = mybir.AluOpType.mult
ADD = mybir.AluOpType.add
DR = mybir.MatmulPerfMode.DoubleRow
HALF = 512
WSC = 32.0  # host scales W by 32 so fp8 sees ~unit-variance values
MSK_OFF = [0]
for _kc in range(1, 8):
    MSK_OFF.append(MSK_OFF[-1] + S - (_kc - 1) * P)

_CACHED = {}


def build_nc(repeats=1):
    nc = bacc.Bacc("TRN2", target_bir_lowering=False, debug=False, num_devices=NCORES)

    xhi_d = nc.dram_tensor("xhi", [P, NCHUNK, S], FP8, kind="ExternalInput").ap()
    xlo_d = nc.dram_tensor("xlo", [P, NCHUNK, S], FP8, kind="ExternalInput").ap()
    wq8_d = {
        hl: nc.dram_tensor(f"wq8{hl}", [NCHUNK, P, 4, 2, P], FP8, kind="ExternalInput").ap()
        for hl in ("h", "l")
    }
    wk8_d = {
        hl: nc.dram_tensor(f"wk8{hl}", [NCHUNK, P, 4, 2, P], FP8, kind="ExternalInput").ap()
        for hl in ("h", "l")
    }
    wv8_d = {
        hl: nc.dram_tensor(f"wv8{hl}", [P, 4, 2, S], FP8, kind="ExternalInput").ap()
        for hl in ("h", "l")
    }
    wo_d = nc.dram_tensor("wo16", [P, NCHUNK, S], BF16, kind="ExternalInput").ap()
    bqk_d = nc.dram_tensor("bqk", [P, 2 * NCHUNK], F32, kind="ExternalInput").ap()
    bv_d = nc.dram_tensor("bv16", [P, D], BF16, kind="ExternalInput").ap()
    bo_d = nc.dram_tensor("bo32", [P, D], F32, kind="ExternalInput").ap()
    msk_d = nc.dram_tensor("msk16", [P, 4608], BF16, kind="ExternalInput").ap()
    out_d = nc.dram_tensor("out", [S, D], F32, kind="ExternalOutput").ap()

    with tile.TileContext(nc) as tc:
        with (
            tc.tile_pool(name="cst", bufs=1) as cstpool,
            tc.tile_pool(name="qk", bufs=3) as qkpool,
            tc.tile_pool(name="v", bufs=8) as vpool,
            tc.tile_pool(name="exp", bufs=6) as exppool,
            tc.tile_pool(name="rbc", bufs=2) as rbcpool,
            tc.tile_pool(name="osb", bufs=2) as osbpool,
            tc.tile_pool(name="big", bufs=2) as bigpool,
            tc.tile_pool(name="pp", bufs=2, space="PSUM") as pp,
            tc.tile_pool(name="po", bufs=4, space="PSUM") as po,
        ):
            for _rep in range(repeats):
                # ---- PE warm-up (no DMA dependency) ----
                warm = cstpool.tile([P, P], BF16, tag="warm")
                nc.vector.memzero(warm[:])
                wps = pp.tile([P, S], F32, tag="pp", name="warmup_ps")
                for wi in range(26):
                    nc.tensor.matmul(wps[:, 0:P], warm[:], warm[:], start=True, stop=True)

                # ---- DMAs, ordered for earliest dependency release ----
                xhi = cstpool.tile([P, NCHUNK, S], FP8, tag="xhi")
                nc.sync.dma_start(xhi[:, 0:4], xhi_d[:, 0:4])
                wq8 = {
                    hl: cstpool.tile([P, NCHUNK, 4, 2, P], FP8, tag=f"wq8{hl}", name=f"wq8{hl}")
                    for hl in ("h", "l")
                }
                wk8 = {
                    hl: cstpool.tile([P, NCHUNK, 4, 2, P], FP8, tag=f"wk8{hl}", name=f"wk8{hl}")
                    for hl in ("h", "l")
                }
                wv8 = {
                    hl: cstpool.tile([P, 4, 2, S], FP8, tag=f"wv8{hl}", name=f"wv8{hl}")
                    for hl in ("h", "l")
                }
                for hl in ("h", "l"):
                    nc.sync.dma_start(wq8[hl][:, 0], wq8_d[hl][0])
                nc.sync.dma_start(xhi[:, 4:8], xhi_d[:, 4:8])
                for hl in ("h", "l"):
                    nc.sync.dma_start(wk8[hl][:, 0], wk8_d[hl][0])
                nc.sync.dma_start(wv8["h"][:], wv8_d["h"][:])
                xlo = cstpool.tile([P, NCHUNK, S], FP8, tag="xlo")
                nc.sync.dma_start(xlo[:, 0:4], xlo_d[:, 0:4])
                nc.sync.dma_start(xlo[:, 4:8], xlo_d[:, 4:8])
                nc.sync.dma_start(wv8["l"][:], wv8_d["l"][:])
                bqk = cstpool.tile([P, 2 * NCHUNK], F32, tag="bqk")
                nc.sync.dma_start(bqk[:], bqk_d[:])
                bv16 = cstpool.tile([P, D], BF16, tag="bv16")
                nc.sync.dma_start(bv16[:], bv_d[:])
                msk = cstpool.tile([P, 4608], BF16, tag="msk")
                nc.sync.dma_start(msk[:], msk_d[:])
                wo16 = cstpool.tile([P, NCHUNK, S], BF16, tag="wo16")
                nc.sync.dma_start(wo16[:], wo_d[:])
                bo32 = cstpool.tile([P, D], F32, tag="bo32")
                nc.sync.dma_start(bo32[:], bo_d[:])

                def dma_wqk_cb(c):
                    for tl, dr in ((wq8, wq8_d), (wk8, wk8_d)):
                        for hl in ("h", "l"):
                            nc.sync.dma_start(tl[hl][:, c], dr[hl][c])

                dma_wqk_cb(1)

                # ---- fp8 DoubleRow projection helpers ----
                def proj_qk(c, w8, bcol, out_tag):
                  with nc.named_scope(f"pj_{out_tag}_{c}"):
                    ps = pp.tile([P, S], F32, tag="pp", name=f"ps_{out_tag}_{c}")
                    for hf in range(2):
                        sl = slice(hf * HALF, (hf + 1) * HALF)
                        seq = [(w8["h"][:, c, j], xhi[:, 2 * j : 2 * j + 2, sl]) for j in range(4)]
                        seq += [(w8["l"][:, c, j], xhi[:, 2 * j : 2 * j + 2, sl]) for j in range(4)]
                        seq += [(w8["h"][:, c, j], xlo[:, 2 * j : 2 * j + 2, sl]) for j in range(4)]
                        for i, (l, r) in enumerate(seq):
                            nc.tensor.matmul(
                                ps[:, sl], l, r,
                                start=(i == 0), stop=(i == len(seq) - 1), perf_mode=DR,
                            )
                    o = qkpool.tile([P, S], F32R, tag=out_tag, name=f"{out_tag}_{c}")
                    nc.vector.tensor_scalar(o[:], ps[:], 1.0 / WSC, bqk[:, bcol + c : bcol + c + 1], MULT, ADD)
                    return o

                def proj_v(sc):
                  with nc.named_scope(f"pj_v_{sc}"):
                    ps = pp.tile([P, S], F32, tag="pp", name=f"ps_v_{sc}")
                    ssl = slice(sc * P, (sc + 1) * P)
                    for hf in range(2):
                        sl = slice(hf * HALF, (hf + 1) * HALF)
                        seq = [(xhi[:, 2 * j : 2 * j + 2, ssl], wv8["h"][:, j, :, sl]) for j in range(4)]
                        seq += [(xhi[:, 2 * j : 2 * j + 2, ssl], wv8["l"][:, j, :, sl]) for j in range(4)]
                        seq += [(xlo[:, 2 * j : 2 * j + 2, ssl], wv8["h"][:, j, :, sl]) for j in range(4)]
                        for i, (l, r) in enumerate(seq):
                            nc.tensor.matmul(
                                ps[:, sl], l, r,
                                start=(i == 0), stop=(i == len(seq) - 1), perf_mode=DR,
                            )
                    vt = vpool.tile([P, H, P], BF16, tag="v", name=f"v_{sc}")
                    # ones in cols 64:128 -> PV psum rows 64:128 hold the
                    # denominator replicated (free: matmul cost is free-size)
                    nc.gpsimd.memset(vt[:], 1.0)
                    for hf in range(2):
                        sl = slice(hf * HALF, (hf + 1) * HALF)
                        nc.scalar.activation(
                            vt[:, hf * 8 : (hf + 1) * 8, 0:DK],
                            ps[:, sl].rearrange("p (h d) -> p h d", h=8),
                            COPY, scale=1.0 / WSC,
                        )
                    nc.vector.tensor_add(
                        vt[:, :, 0:DK],
                        vt[:, :, 0:DK],
                        bv16[:].rearrange("p (h d) -> p h d", h=H),
                    )
                    return vt

                # ---- attention pieces ----
                qT, kT, vtiles = [None] * NCHUNK, [None] * NCHUNK, [None] * NCHUNK
                attn = [None, None]

                def scores_exp(h, kc):
                    nm = nc.named_scope(f"sc_{h}_{kc}")
                    nm.__enter__()
                    c, r = h // 2, (h % 2) * DK
                    ps = pp.tile([P, S], F32, tag="pp", name=f"pss_{h}_{kc}")
                    lhs = kT[c][r : r + DK, kc * P : (kc + 1) * P]
                    for hf in range(2):
                        sl = slice(hf * HALF, (hf + 1) * HALF)
                        nc.tensor.matmul(ps[:, sl], lhs, qT[c][r : r + DK, sl], start=True, stop=True)
                    et = exppool.tile([P, S], BF16, tag="et", name=f"et_{h}_{kc}")
                    nc.scalar.activation(et[:], ps[:], EXP)
                    w = S - kc * P
                    off = MSK_OFF[kc]
                    nc.vector.tensor_mul(et[:, kc * P : S], et[:, kc * P : S], msk[:, off : off + w])
                    nm.__exit__(None, None, None)
                    return et

                def emit_pv(h, kc, pso, et):
                  with nc.named_scope(f"pv_{h}_{kc}"):
                    for hf in range(2):
                        sl = slice(hf * HALF, (hf + 1) * HALF)
                        nc.tensor.matmul(
                            pso[hf][:, :], vtiles[kc][:, h, :], et[:, sl],
                            start=(kc == 0), stop=(kc == NCHUNK - 1),
                        )

                def emit_norm(h, pso, hf):
                  with nc.named_scope(f"norm_{h}_{hf}"):
                    ph = pso[hf]
                    rbc = rbcpool.tile([DK, HALF], F32, tag="rbc", name=f"rbc_{h}_{hf}")
                    nc.vector.reciprocal(rbc[:], ph[DK : 2 * DK, :])
                    # q in [hf*512, (hf+1)*512) -> u = q//16 in [hf*32, (hf+1)*32)
                    src = ph[0:DK, :].rearrange("d (u j) -> d j u", j=16)
                    rbs = rbc[:].rearrange("d (u j) -> d j u", j=16)
                    usl = slice(h * DK + hf * 32, h * DK + (hf + 1) * 32)
                    for g in range(2):
                        if attn[g] is None:
                            attn[g] = bigpool.tile([P, 4, S], BF16, tag="big", name=f"attnq_{g}")
                        for e in range(2):
                            jsl = slice(8 * g + e, 8 * (g + 1), 2)
                            nc.vector.tensor_mul(
                                attn[g][e * DK : (e + 1) * DK, :, usl],
                                src[:, jsl, :], rbs[:, jsl, :],
                            )

                osb_cur = [None]

                def emit_oproj_half(sc, hf):
                  with nc.named_scope(f"oproj_{sc}_{hf}"):
                    if hf == 0:
                        osb_cur[0] = osbpool.tile([P, S], F32, tag="osb", name=f"ot_{sc}")
                    ot = osb_cur[0]
                    sl = slice(hf * HALF, (hf + 1) * HALF)
                    ps = po.tile([P, HALF], F32, tag="po", name=f"psf_{sc}_{hf}")
                    for cc in range(NCHUNK):
                        nc.tensor.matmul(
                            ps[:],
                            attn[cc // 4][:, cc % 4, sc * P : (sc + 1) * P],
                            wo16[:, cc, sl],
                            start=(cc == 0), stop=(cc == NCHUNK - 1),
                        )
                    nc.vector.tensor_add(ot[:, sl], ps[:], bo32[:, sl])
                    nc.sync.dma_start(out_d[sc * P : (sc + 1) * P, sl], ot[:, sl])

                def emit_oproj(sc):
                    emit_oproj_half(sc, 0)
                    emit_oproj_half(sc, 1)

                # ---- pre-phase: qk chunk 0 + first v chunks ----
                qT[0] = proj_qk(0, wq8, 0, "qT")
                kT[0] = proj_qk(0, wk8, NCHUNK, "kT")
                for sc in range(4):
                    vtiles[sc] = proj_v(sc)

                # ---- pair loop (pend carried across pair boundaries) ----
                from collections import deque

                pend = deque()

                def pop_pv():
                    ph, pkc, ppso, pet = pend.popleft()
                    emit_pv(ph, pkc, ppso, pet)
                    if pkc == NCHUNK - 1:
                        emit_norm(ph, ppso, 0)
                        emit_norm(ph, ppso, 1)

                for c in range(NCHUNK):
                    h0, h1 = 2 * c, 2 * c + 1
                    if c < NCHUNK - 2:
                        dma_wqk_cb(c + 2)
                    pso0 = [po.tile([P, HALF], F32, tag="po", name=f"pso_{h0}_{hf}") for hf in range(2)]
                    for kc in range(NCHUNK):
                        et = scores_exp(h0, kc)
                        if c == 0 and kc % 2 == 1:
                            vtiles[4 + kc // 2] = proj_v(4 + kc // 2)
                        # eagerly drain PVs of previous heads; keep lag 4 for own
                        if len(pend) >= 4 or (pend and pend[0][0] < h0):
                            pop_pv()
                        pend.append((h0, kc, pso0, et))
                    if c == NCHUNK - 1:
                        emit_oproj(NCHUNK - 2)
                    pso1 = [po.tile([P, HALF], F32, tag="po", name=f"pso_{h1}_{hf}") for hf in range(2)]
                    for kc in range(NCHUNK):
                        et = scores_exp(h1, kc)
                        if len(pend) >= 4 or (pend and pend[0][0] < h0):
                            pop_pv()
                        if c == NCHUNK - 1 and pend:
                            pop_pv()
                        pend.append((h1, kc, pso1, et))
                    if c < NCHUNK - 1:
                        qT[c + 1] = proj_qk(c + 1, wq8, 0, "qT")
                        pop_pv()
                        pop_pv()
                        kT[c + 1] = proj_qk(c + 1, wk8, NCHUNK, "kT")
                        while len(pend) > 3:
                            pop_pv()
                        if c >= 1:
                            emit_oproj(c - 1)
                    else:
                        while pend:
                            pop_pv()
                emit_oproj(NCHUNK - 1)

    nc.compile()
    return nc


def _host_masks(prefix_b: int):
    """Multiplicative 0/1 mask (bf16) applied to exp output."""
    i = np.arange(P)[:, None]
    segs = []
    for kc in range(NCHUNK):
        q = np.arange(kc * P, S)[None, :]
        k = kc * P + i
        allowed = (q < prefix_b) | (k >= q)
        segs.append(allowed.astype(np.float32))
    return np.concatenate(segs, axis=1).astype(ml_dtypes.bfloat16)


def _split8(a):
    hi = a.astype(ml_dtypes.float8_e4m3fn)
    lo = (a - hi.astype(np.float32)).astype(ml_dtypes.float8_e4m3fn)
    return hi, lo


def _pack_wqk(w):
    """[8cb, 128k, 4j, 2t, 128m] from W32 [(2j+t)*128+k, cb*128+m]."""
    a = (w * WSC).reshape(4, 2, P, NCHUNK, P).transpose(3, 2, 0, 1, 4)
    return _split8(np.ascontiguousarray(a))


def _pack_wv(w):
    """[128k, 4j, 2t, 1024n] from Wv32 [(2j+t)*128+k, n]."""
    a = (w * WSC).reshape(4, 2, P, S).transpose(2, 0, 1, 3)
    return _split8(np.ascontiguousarray(a))


def kernel(x, prefix, Wq, bq, Wk, bk, Wv, bv, Wo, bo, _trace=False):
    x = np.asarray(x, dtype=np.float32)
    prefix = np.asarray(prefix)
    Wq, Wk, Wv, Wo = (np.asarray(w, np.float32) for w in (Wq, Wk, Wv, Wo))
    bqk = np.stack(
        [np.asarray(bq, np.float32).reshape(NCHUNK, P), np.asarray(bk, np.float32).reshape(NCHUNK, P)],
        axis=0,
    ).reshape(2 * NCHUNK, P).T.copy()  # [128, 16]: cols 0-7 bq chunks, 8-15 bk

    wq8h, wq8l = _pack_wqk(Wq)
    wk8h, wk8l = _pack_wqk(Wk)
    wv8h, wv8l = _pack_wv(Wv)
    wo16 = np.ascontiguousarray(
        Wo.reshape(NCHUNK, P, S).transpose(1, 0, 2)
    ).astype(ml_dtypes.bfloat16)
    bv16 = np.broadcast_to(np.asarray(bv, np.float32).reshape(1, D), (P, D)).astype(ml_dtypes.bfloat16)
    bo32 = np.broadcast_to(np.asarray(bo, np.float32).reshape(1, D), (P, D)).astype(np.float32).copy()

    if "nc" not in _CACHED:
        _CACHED["nc"] = build_nc()
    nc = _CACHED["nc"]

    in_maps = []
    for b in range(B):
        xt = np.ascontiguousarray(x[b].T)  # [D, S]
        xts = np.ascontiguousarray(xt.reshape(NCHUNK, P, S).transpose(1, 0, 2))  # [128, 8, 1024]
        xhi, xlo = _split8(xts)
        mask16 = _host_masks(int(prefix[b]))
        in_maps.append(
            {
                "xhi": xhi, "xlo": xlo,
                "wq8h": wq8h, "wq8l": wq8l,
                "wk8h": wk8h, "wk8l": wk8l,
                "wv8h": wv8h, "wv8l": wv8l,
                "wo16": wo16, "bqk": bqk, "bv16": bv16, "bo32": bo32,
                "msk16": mask16,
            }
        )

    res = run_bass_kernel_spmd(nc, in_maps, core_ids=list(range(NCORES)), trace=_trace)
    out = np.stack([res.results[b]["out"] for b in range(B)], axis=0)
    if _trace:
        return out, res
    return out
